# revision 21
# baseline (speedup 1.0000x reference)
"""Clustered Linformer Attention — TRN2 Bass kernel, batch-parallel over 8 NeuronCores.

Per core (one batch element b), all matmuls in bf16 (inputs pre-rounded on host):
  A:  x^T loaded via DMA-transpose; q^T = wq^T x^T ; k, v natural (PE)
  B:  kp^T/vp^T accumulated in pinned PSUM across all n-groups; the two heads of
      each pair run col-tiled (M=64 at col 0 / col 64) and concurrently.
  C:  scores^T_h = kp_h q_h^T / 8 -> exp (ACT). K=64 row-tiled: the head pair
      runs concurrently on PE row-groups 0:63 / 64:127.
  F:  out_raw^T_h = vp_h-contraction over r; softmax denominators ride along via
      a ones column in the stationary (row oro of the fop output).
  N:  per-pair one-hot matmuls broadcast the two sum rows across partitions
      (col-tiled, concurrent), reciprocal fused with the PSUM read (DVE),
      normalize on gpsimd.
  G:  y = concat @ w_dense, DMA'd straight from PSUM; bias added on host.
"""
import sys
import numpy as np

for _p in ("/opt/trn_rl_repo", "/root/.axon_site/_ro/trn_rl_repo"):
    if _p not in sys.path:
        sys.path.insert(0, _p)

import concourse.bacc as bacc
import concourse.tile as tile
from concourse import mybir
from concourse.bass_utils import run_bass_kernel_spmd

B, N, D = 8, 4096, 512
H, R = 8, 256
DEP = D // H          # 64
P = 128
NG = 8                # n-groups for phase A/B
GN = N // NG          # 512 rows per group
NS = 8                # n-strips for phase C..G
SN = N // NS          # 512 cols per strip
F32 = mybir.dt.float32
BF16 = mybir.dt.bfloat16
EXPF = mybir.ActivationFunctionType.Exp

_cache = {}


def build_program(repeat=1, dbg=False):
    key = ("nc", repeat, dbg)
    if key in _cache:
        return _cache[key]
    nc = bacc.Bacc("TRN2", target_bir_lowering=False, debug=False)
    x = nc.dram_tensor("x", [N, D], BF16, kind="ExternalInput").ap()
    wq = nc.dram_tensor("wq", [D, D], BF16, kind="ExternalInput").ap()
    wk = nc.dram_tensor("wk", [D, D], BF16, kind="ExternalInput").ap()
    wv = nc.dram_tensor("wv", [D, D], BF16, kind="ExternalInput").ap()
    wd = nc.dram_tensor("wd", [D, D], BF16, kind="ExternalInput").ap()
    E = nc.dram_tensor("E", [H, N, R], BF16, kind="ExternalInput").ap()
    Fm = nc.dram_tensor("F", [H, N, R], BF16, kind="ExternalInput").ap()
    ident_in = nc.dram_tensor("ident", [P, P], BF16, kind="ExternalInput").ap()
    y = nc.dram_tensor("y", [N, D], F32, kind="ExternalOutput").ap()
    if dbg:
        dqT = nc.dram_tensor("dqT", [P, N], BF16, kind="ExternalOutput").ap()
        dkpP = nc.dram_tensor("dkpP", [P, R], BF16, kind="ExternalOutput").ap()
        dvp2 = nc.dram_tensor("dvp2", [2, P, P], BF16, kind="ExternalOutput").ap()
        dexp = nc.dram_tensor("dexp", [P, SN], BF16, kind="ExternalOutput").ap()
        dstag = nc.dram_tensor("dstag", [2, P, SN], BF16, kind="ExternalOutput").ap()
        dbcs = nc.dram_tensor("dbcs", [P, SN], F32, kind="ExternalOutput").ap()
        dcsR = nc.dram_tensor("dcsR", [P, 4, SN], BF16, kind="ExternalOutput").ap()
        dkg = nc.dram_tensor("dkg", [P, D], BF16, kind="ExternalOutput").ap()
        dvg = nc.dram_tensor("dvg", [P, D], BF16, kind="ExternalOutput").ap()

    with tile.TileContext(nc) as tc, nc.allow_low_precision(reason="bf16 kernel"):
      for _rep in range(repeat):
        with tc.tile_pool(name="outer", bufs=1) as po:
            # ---- persistent tiles ----
            xT = [po.tile([P, N], BF16, tag=f"xT{c}", name=f"xT{c}") for c in range(4)]
            qT = [po.tile([P, N], BF16, tag=f"qT{c}", name=f"qT{c}") for c in range(4)]
            kpP = [po.tile([P, R], BF16, tag=f"kpP{p}", name=f"kpP{p}") for p in range(4)]
            vp2 = [[po.tile([P, P], BF16, tag=f"vp2_{h}_{rc}", name=f"vp2_{h}_{rc}")
                    for rc in range(2)] for h in range(H)]
            wq_t = [po.tile([P, D], BF16, tag=f"wq{c}", name=f"wq{c}") for c in range(4)]
            wk_t = [po.tile([P, D], BF16, tag=f"wk{c}", name=f"wk{c}") for c in range(4)]
            wv_t = [po.tile([P, D], BF16, tag=f"wv{c}", name=f"wv{c}") for c in range(4)]
            wd_t = [po.tile([P, D], BF16, tag=f"wd{c}", name=f"wd{c}") for c in range(4)]
            S0 = po.tile([P, P], BF16, tag="S0", name="S0")
            S1 = po.tile([P, P], BF16, tag="S1", name="S1")
            ident = po.tile([P, P], BF16, tag="ident", name="ident")

            nc.sync.dma_start(ident[:], ident_in)
            for c in range(4):
                nc.sync.dma_start(wq_t[c][:], wq[c * P:(c + 1) * P, :])
                nc.sync.dma_start(wk_t[c][:], wk[c * P:(c + 1) * P, :])
                nc.sync.dma_start(wv_t[c][:], wv[c * P:(c + 1) * P, :])
                nc.sync.dma_start(wd_t[c][:], wd[c * P:(c + 1) * P, :])
            # one-hot broadcast stationaries (full M=128, used as a 2-matmul
            # accumulation group): out rows 0:64 <- in row 64 (head0 sums),
            # out rows 64:128 <- in row 0 (head1 sums)
            nc.gpsimd.memset(S0[:], 0.0)
            nc.gpsimd.memset(S1[:], 0.0)
            nc.gpsimd.memset(S0[DEP:DEP + 1, 0:DEP], 1.0)
            nc.gpsimd.memset(S1[0:1, DEP:P], 1.0)
            # vp2: zeros + single ones column at oro (sum extractor)
            for h in range(H):
                oro = DEP * (1 - h % 2)
                for rc in range(2):
                    nc.gpsimd.memset(vp2[h][rc][:], 0.0)
                    nc.gpsimd.memset(vp2[h][rc][:, oro:oro + 1], 1.0)

            # ================= PHASE A+B =================
            with tc.tile_pool(name="psA", bufs=2, space="PSUM") as psA, \
                 tc.tile_pool(name="psB", bufs=1, space="PSUM") as psB, \
                 tc.tile_pool(name="pkv", bufs=6) as pkv, \
                 tc.tile_pool(name="pef", bufs=6) as pef:

                # one full PSUM bank per pair: [:, 0, :] = kp^T, [:, 1, :] = vp^T
                # 4 disjoint accumulation streams share each bank, so no
                # matmul may use start=True (bank-wide has_written clear);
                # instead the bank is zeroed once and every matmul accumulates.
                kv_ps = [psB.tile([P, 2, R], F32, tag=f"kv_ps{p}", name=f"kv_ps{p}")
                         for p in range(4)]
                for p in range(4):
                    nc.vector.memset(kv_ps[p][:], 0.0)

                # prefetch g=0 x^T slabs
                for c in range(4):
                    nc.sync.dma_start_transpose(
                        xT[c][:, 0:GN], x[0:GN, c * P:(c + 1) * P])

                for g in range(NG):
                    n0 = g * GN
                    # prefetch next group's x^T ahead of this group's E/F
                    if g + 1 < NG:
                        m0 = (g + 1) * GN
                        for c in range(4):
                            nc.sync.dma_start_transpose(
                                xT[c][:, m0:m0 + GN], x[m0:m0 + GN, c * P:(c + 1) * P])
                    # q^T
                    for dq in range(4):
                        qp = psA.tile([P, GN], F32, tag="qp", name="qp")
                        for c in range(4):
                            nc.tensor.matmul(
                                qp[:], wq_t[c][:, dq * P:(dq + 1) * P],
                                xT[c][:, n0:n0 + GN],
                                start=(c == 0), stop=(c == 3))
                        nc.scalar.copy(qT[dq][:, n0:n0 + GN], qp[:])
                    # k, v natural
                    kg = [pkv.tile([P, D], BF16, tag="kg", name="kg") for i in range(4)]
                    vg = [pkv.tile([P, D], BF16, tag="vg", name="vg") for i in range(4)]
                    for i in range(4):
                        kp_ = psA.tile([P, D], F32, tag="qp", name="qp")
                        for c in range(4):
                            nc.tensor.matmul(
                                kp_[:], xT[c][:, n0 + i * P:n0 + (i + 1) * P],
                                wk_t[c][:], start=(c == 0), stop=(c == 3))
                        nc.scalar.copy(kg[i][:], kp_[:])
                        vp_ = psA.tile([P, D], F32, tag="qp", name="qp")
                        for c in range(4):
                            nc.tensor.matmul(
                                vp_[:], xT[c][:, n0 + i * P:n0 + (i + 1) * P],
                                wv_t[c][:], start=(c == 0), stop=(c == 3))
                        nc.vector.tensor_copy(vg[i][:], vp_[:])
                        if dbg and g == 0 and i == 0:
                            nc.sync.dma_start(dkg, kg[0][:])
                            nc.sync.dma_start(dvg, vg[0][:])
                    # B: project k, v through E_h, F_h; M=64 per head, the two
                    # parities col-tiled at (0,0)/(0,64) run concurrently;
                    # accumulation lives in PSUM across all groups.
                    for p in range(4):
                        for par in range(2):
                            h = 2 * p + par
                            ro = DEP * par
                            Eh = pef.tile([P, 4, R], BF16, tag="ef", name="ef")
                            nc.sync.dma_start(
                                Eh[:], E[h, n0:n0 + GN, :].rearrange(
                                    "(i p) r -> p i r", p=P))
                            Fh = pef.tile([P, 4, R], BF16, tag="ef", name="ef")
                            nc.sync.dma_start(
                                Fh[:], Fm[h, n0:n0 + GN, :].rearrange(
                                    "(i p) r -> p i r", p=P))
                            for i in range(4):
                                nc.tensor.matmul(
                                    kv_ps[p][ro:ro + DEP, 0, :],
                                    kg[i][:, p * P + ro:p * P + ro + DEP],
                                    Eh[:, i, :],
                                    start=False,
                                    stop=(g == NG - 1 and i == 3),
                                    skip_group_check=True)
                            for i in range(4):
                                nc.tensor.matmul(
                                    kv_ps[p][ro:ro + DEP, 1, :],
                                    vg[i][:, p * P + ro:p * P + ro + DEP],
                                    Fh[:, i, :],
                                    start=False,
                                    stop=(g == NG - 1 and i == 3),
                                    skip_group_check=True)

                # evict kp to bf16 stationaries; transpose vp pairs into vp2
                for p in range(4):
                    nc.vector.tensor_copy(kpP[p][:], kv_ps[p][:, 0, :])
                    vpS = pkv.tile([P, R], BF16, tag="vpS", name="vpS")
                    nc.vector.tensor_copy(vpS[:], kv_ps[p][:, 1, :])
                    if dbg and p == 0:
                        nc.sync.dma_start(dkpP, kpP[0][:])
                        nc.sync.dma_start(dqT, qT[0][:])
                    for rc in range(2):
                        vt = psA.tile([P, P], BF16, tag="vt", name="vt")
                        nc.tensor.transpose(vt[:], vpS[:, rc * P:(rc + 1) * P],
                                            ident[:])
                        for par in range(2):
                            h = 2 * p + par
                            ro = DEP * par
                            nc.vector.tensor_copy(
                                vp2[h][rc][:, ro:ro + DEP], vt[:, ro:ro + DEP])

            # ================= PHASE C..G =================
            with tc.tile_pool(name="psc", bufs=4, space="PSUM") as psc, \
                 tc.tile_pool(name="psf", bufs=2, space="PSUM") as psf, \
                 tc.tile_pool(name="psb", bufs=1, space="PSUM") as psb, \
                 tc.tile_pool(name="psy", bufs=1, space="PSUM") as psy, \
                 tc.tile_pool(name="pexp", bufs=6) as pexp, \
                 tc.tile_pool(name="pstag", bufs=10) as pstag, \
                 tc.tile_pool(name="pbc", bufs=4) as pbc, \
                 tc.tile_pool(name="pys", bufs=3) as pys, \
                 tc.tile_pool(name="pcs", bufs=2) as pcs:
                for s in range(NS):
                    c0 = s * SN
                    csR = pcs.tile([P, 4, SN], BF16, tag="csR", name="csR")
                    for p in range(4):
                        stags = {}
                        for par in range(2):
                            h = 2 * p + par
                            ro = DEP * par
                            expT = [pexp.tile([P, SN], BF16, tag="expT",
                                              name="expT") for rc in range(2)]
                            for rc in range(2):
                                scp = psc.tile([P, SN], F32, tag="sc", name="sc")
                                nc.tensor.matmul(
                                    scp[:],
                                    kpP[p][ro:ro + DEP, rc * P:(rc + 1) * P],
                                    qT[p][ro:ro + DEP, c0:c0 + SN],
                                    start=True, stop=True)
                                nc.scalar.activation(
                                    expT[rc][:], scp[:], EXPF,
                                    scale=float(1.0 / np.sqrt(np.float32(DEP))))
                            fop = psf.tile([P, SN], F32, tag="fo", name="fo")
                            for rc in range(2):
                                nc.tensor.matmul(
                                    fop[:], vp2[h][rc][:], expT[rc][:],
                                    start=(rc == 0), stop=(rc == 1))
                            stag = pstag.tile([P, SN], BF16, tag="stag",
                                              name="stag")
                            nc.vector.tensor_copy(stag[:], fop[:])
                            stags[par] = stag
                            if dbg and s == 0 and h == 0:
                                nc.sync.dma_start(dexp, expT[0][:])
                                for rc in range(2):
                                    nc.sync.dma_start(dvp2[rc], vp2[0][rc][:])
                            if dbg and s == 0 and p == 0:
                                nc.sync.dma_start(dstag[par], stag[:])
                        # broadcast the two sum rows across partitions
                        # (col-tiled one-hot matmuls), reciprocal fused w/ evict
                        bcp = psb.tile([P, SN], F32, tag="bc", name="bc")
                        nc.tensor.matmul(bcp[:], S0[:], stags[0][:],
                                         start=True, stop=False)
                        nc.tensor.matmul(bcp[:], S1[:], stags[1][:],
                                         start=False, stop=True)
                        bcs = pbc.tile([P, SN], F32, tag="bcs", name="bcs")
                        nc.vector.reciprocal(bcs[:], bcp[:])
                        if dbg and s == 0 and p == 0:
                            nc.sync.dma_start(dbcs, bcs[:])
                        for par in range(2):
                            ro = DEP * par
                            nc.gpsimd.tensor_mul(
                                csR[ro:ro + DEP, p, :],
                                stags[par][ro:ro + DEP, :],
                                bcs[ro:ro + DEP, :])
                    if dbg and s == 0:
                        nc.sync.dma_start(dcsR, csR[:])
                    for j in range(4):
                        yp = psy.tile([P, D], F32, tag="y", name="y")
                        for c in range(4):
                            nc.tensor.matmul(
                                yp[:], csR[:, c, j * P:(j + 1) * P], wd_t[c][:],
                                start=(c == 0), stop=(c == 3))
                        ys = pys.tile([P, D], F32, tag="ys", name="ys")
                        nc.vector.tensor_copy(ys[:], yp[:])
                        nc.sync.dma_start(y[c0 + j * P:c0 + (j + 1) * P, :], ys[:])

    nc.compile()
    _cache[key] = nc
    return nc


_BF = mybir.dt.np(BF16)


def make_in_maps(x, wq, wk, wv, E, F, w_dense, b_dense):
    x = np.ascontiguousarray(np.asarray(x, dtype=np.float32)).astype(_BF)
    consts = {
        "wq": np.ascontiguousarray(np.asarray(wq, np.float32)).astype(_BF),
        "wk": np.ascontiguousarray(np.asarray(wk, np.float32)).astype(_BF),
        "wv": np.ascontiguousarray(np.asarray(wv, np.float32)).astype(_BF),
        "wd": np.ascontiguousarray(np.asarray(w_dense, np.float32)).astype(_BF),
        "E": np.ascontiguousarray(np.asarray(E, np.float32)).astype(_BF),
        "F": np.ascontiguousarray(np.asarray(F, np.float32)).astype(_BF),
        "ident": np.eye(P, dtype=np.float32).astype(_BF),
    }
    return [{"x": x[b], **consts} for b in range(B)]


def kernel(x, wq, wk, wv, E, F, w_dense, b_dense):
    nc = build_program()
    in_maps = make_in_maps(x, wq, wk, wv, E, F, w_dense, b_dense)
    res = run_bass_kernel_spmd(nc, in_maps, list(range(B)))
    out = np.stack([res.results[b]["y"] for b in range(B)], axis=0)
    out = out.astype(np.float32) + np.asarray(b_dense, np.float32)[None, None, :]
    return out


# revision 40
# speedup vs baseline: 1.3748x; 1.3748x over previous
"""Clustered Linformer Attention — TRN2 Bass kernel, batch-parallel over 8 NeuronCores.

Per core (one batch element b), all matmuls in bf16 (inputs pre-rounded on host):
  A:  x^T loaded via DMA-transpose; q^T = wq^T x^T ; k, v natural (PE)
  B:  kp^T/vp^T accumulated in pinned PSUM across all n-groups; the two heads of
      each pair run col-tiled (M=64 at col 0 / col 64) and concurrently.
  C:  scores^T_h = kp_h q_h^T / 8 -> exp (ACT). K=64 row-tiled: the head pair
      runs concurrently on PE row-groups 0:63 / 64:127.
  F:  out_raw^T_h = vp_h-contraction over r; softmax denominators ride along via
      a ones column in the stationary (row oro of the fop output).
  N:  per-pair one-hot matmuls broadcast the two sum rows across partitions
      (col-tiled, concurrent), reciprocal fused with the PSUM read (DVE),
      normalize on gpsimd.
  G:  y = concat @ w_dense, DMA'd straight from PSUM; bias added on host.
"""
import sys
import numpy as np

for _p in ("/opt/trn_rl_repo", "/root/.axon_site/_ro/trn_rl_repo"):
    if _p not in sys.path:
        sys.path.insert(0, _p)

import concourse.bacc as bacc
import concourse.tile as tile
from concourse import mybir
from concourse.bass_utils import run_bass_kernel_spmd

B, N, D = 8, 4096, 512
H, R = 8, 256
DEP = D // H          # 64
P = 128
NG = 8                # n-groups for phase A/B
GN = N // NG          # 512 rows per group
NS = 8                # n-strips for phase C..G
SN = N // NS          # 512 cols per strip
F32 = mybir.dt.float32
BF16 = mybir.dt.bfloat16
EXPF = mybir.ActivationFunctionType.Exp

_cache = {}


def build_program(repeat=1, dbg=False, phase="full"):
    key = ("nc", repeat, dbg, phase)
    if key in _cache:
        return _cache[key]
    nc = bacc.Bacc("TRN2", target_bir_lowering=False, debug=False)
    # x arrives pre-transposed [D, N]; E/F pre-packed [H, NG, P, 4, R] so every
    # DMA is a fully-contiguous per-partition block (host-side numpy prep).
    x = nc.dram_tensor("x", [D, N], BF16, kind="ExternalInput").ap()
    wq = nc.dram_tensor("wq", [D, D], BF16, kind="ExternalInput").ap()
    wk = nc.dram_tensor("wk", [D, D], BF16, kind="ExternalInput").ap()
    wv = nc.dram_tensor("wv", [D, D], BF16, kind="ExternalInput").ap()
    wd = nc.dram_tensor("wd", [D, D], BF16, kind="ExternalInput").ap()
    # E and F interleaved per (group, pair): one DMA delivers both parities of
    # both projections: [g, p, P, ef(2), par(2), i(4), r]
    EF = nc.dram_tensor("EF", [NG, 4, P, 2, 2, 4, R], BF16,
                        kind="ExternalInput").ap()
    ident_in = nc.dram_tensor("ident", [P, P], BF16, kind="ExternalInput").ap()
    y = nc.dram_tensor("y", [N, D], F32, kind="ExternalOutput").ap()
    if dbg:
        dqT = nc.dram_tensor("dqT", [P, N], BF16, kind="ExternalOutput").ap()
        dkpP = nc.dram_tensor("dkpP", [P, R], BF16, kind="ExternalOutput").ap()
        dvp2 = nc.dram_tensor("dvp2", [2, P, P], BF16, kind="ExternalOutput").ap()
        dexp = nc.dram_tensor("dexp", [P, SN], BF16, kind="ExternalOutput").ap()
        dstag = nc.dram_tensor("dstag", [2, P, SN], BF16, kind="ExternalOutput").ap()
        dbcs = nc.dram_tensor("dbcs", [P, SN], F32, kind="ExternalOutput").ap()
        dcsR = nc.dram_tensor("dcsR", [P, 4, SN], BF16, kind="ExternalOutput").ap()
        dkg = nc.dram_tensor("dkg", [P, D], BF16, kind="ExternalOutput").ap()
        dvg = nc.dram_tensor("dvg", [P, D], BF16, kind="ExternalOutput").ap()

    with tile.TileContext(nc) as tc, nc.allow_low_precision(reason="bf16 kernel"):
      for _rep in range(repeat):
        with tc.tile_pool(name="outer", bufs=1) as po:
            # ---- persistent tiles ----
            xT = [po.tile([P, N], BF16, tag=f"xT{c}", name=f"xT{c}") for c in range(4)]
            qT = [po.tile([P, N], BF16, tag=f"qT{c}", name=f"qT{c}") for c in range(4)]
            kpP = [po.tile([P, R], BF16, tag=f"kpP{p}", name=f"kpP{p}") for p in range(4)]
            vp2 = [[po.tile([P, P], BF16, tag=f"vp2_{h}_{rc}", name=f"vp2_{h}_{rc}")
                    for rc in range(2)] for h in range(H)]
            wq_t = [po.tile([P, D], BF16, tag=f"wq{c}", name=f"wq{c}") for c in range(4)]
            wk_t = [po.tile([P, D], BF16, tag=f"wk{c}", name=f"wk{c}") for c in range(4)]
            wv_t = [po.tile([P, D], BF16, tag=f"wv{c}", name=f"wv{c}") for c in range(4)]
            wd_t = [po.tile([P, D], BF16, tag=f"wd{c}", name=f"wd{c}") for c in range(4)]
            S0 = po.tile([P, P], BF16, tag="S0", name="S0")
            S1 = po.tile([P, P], BF16, tag="S1", name="S1")
            ident = po.tile([P, P], BF16, tag="ident", name="ident")

            nc.sync.dma_start(ident[:], ident_in)
            for c in range(4):
                nc.sync.dma_start(wq_t[c][:], wq[c * P:(c + 1) * P, :])
                nc.sync.dma_start(wk_t[c][:], wk[c * P:(c + 1) * P, :])
                nc.sync.dma_start(wv_t[c][:], wv[c * P:(c + 1) * P, :])
                nc.sync.dma_start(wd_t[c][:], wd[c * P:(c + 1) * P, :])
            # one-hot broadcast stationaries (full M=128, used as a 2-matmul
            # accumulation group): out rows 0:64 <- in row 64 (head0 sums),
            # out rows 64:128 <- in row 0 (head1 sums)
            nc.gpsimd.memset(S0[:], 0.0)
            nc.gpsimd.memset(S1[:], 0.0)
            nc.gpsimd.memset(S0[DEP:DEP + 1, 0:DEP], 1.0)
            nc.gpsimd.memset(S1[0:1, DEP:P], 1.0)
            # vp2: zeros + single ones column at oro (sum extractor)
            for h in range(H):
                oro = DEP * (1 - h % 2)
                for rc in range(2):
                    nc.gpsimd.memset(vp2[h][rc][:], 0.0)
                    nc.gpsimd.memset(vp2[h][rc][:, oro:oro + 1], 1.0)

            # ================= PHASE A+B =================
            with tc.tile_pool(name="psA", bufs=2, space="PSUM") as psA, \
                 tc.tile_pool(name="psB", bufs=1, space="PSUM") as psB, \
                 tc.tile_pool(name="pkv", bufs=6) as pkv, \
                 tc.tile_pool(name="pef", bufs=5) as pef:

                # one full PSUM bank per pair: [:, 0, :] = kp^T, [:, 1, :] = vp^T
                # 4 disjoint accumulation streams share each bank, so no
                # matmul may use start=True (bank-wide has_written clear);
                # instead the bank is zeroed once and every matmul accumulates.
                kv_ps = [psB.tile([P, 2, R], F32, tag=f"kv_ps{p}", name=f"kv_ps{p}")
                         for p in range(4)]
                for p in range(4):
                    nc.vector.memset(kv_ps[p][:], 0.0)

                # prefetch g=0/1 x^T slabs
                for c in range(4):
                    nc.sync.dma_start(
                        xT[c][:, 0:2 * GN], x[c * P:(c + 1) * P, 0:2 * GN])

                for g in range(NG):
                    n0 = g * GN
                    # prefetch x^T two groups ahead of this group's E/F
                    if g + 2 < NG and g % 2 == 0:
                        m0 = (g + 2) * GN
                        for c in range(4):
                            nc.sync.dma_start(
                                xT[c][:, m0:m0 + 2 * GN],
                                x[c * P:(c + 1) * P, m0:m0 + 2 * GN])
                    # q^T
                    for dq in range(4):
                        qp = psA.tile([P, GN], F32, tag="qp", name="qp")
                        for c in range(4):
                            nc.tensor.matmul(
                                qp[:], wq_t[c][:, dq * P:(dq + 1) * P],
                                xT[c][:, n0:n0 + GN],
                                start=(c == 0), stop=(c == 3))
                        nc.scalar.copy(qT[dq][:, n0:n0 + GN], qp[:])
                    # k, v natural
                    kg = [pkv.tile([P, D], BF16, tag="kg", name="kg") for i in range(4)]
                    vg = [pkv.tile([P, D], BF16, tag="vg", name="vg") for i in range(4)]
                    for i in range(4):
                        # k and v interleaved per c-chunk: consecutive matmuls
                        # share the same stationary xT slice
                        kp_ = psA.tile([P, D], F32, tag="qp", name="qp")
                        vp_ = psA.tile([P, D], F32, tag="qp", name="qp")
                        for c in range(4):
                            nc.tensor.matmul(
                                kp_[:], xT[c][:, n0 + i * P:n0 + (i + 1) * P],
                                wk_t[c][:], start=(c == 0), stop=(c == 3))
                            nc.tensor.matmul(
                                vp_[:], xT[c][:, n0 + i * P:n0 + (i + 1) * P],
                                wv_t[c][:], start=(c == 0), stop=(c == 3))
                        nc.scalar.copy(kg[i][:], kp_[:])
                        nc.vector.tensor_copy(vg[i][:], vp_[:])
                        if dbg and g == 0 and i == 0:
                            nc.sync.dma_start(dkg, kg[0][:])
                            nc.sync.dma_start(dvg, vg[0][:])
                    # B: project k, v through E_h, F_h; M=64 per head, the two
                    # parities col-tiled at (0,0)/(0,64) run concurrently;
                    # accumulation lives in PSUM across all groups.
                    for p in range(4):
                        eft = pef.tile([P, 2, 2, 4, R], BF16, tag="ef", name="ef")
                        nc.sync.dma_start(eft[:], EF[g, p])
                        for par in range(2):
                            ro = DEP * par
                            for i in range(4):
                                nc.tensor.matmul(
                                    kv_ps[p][ro:ro + DEP, 0, :],
                                    kg[i][:, p * P + ro:p * P + ro + DEP],
                                    eft[:, 0, par, i, :],
                                    start=False,
                                    stop=(g == NG - 1 and i == 3),
                                    skip_group_check=True)
                            for i in range(4):
                                nc.tensor.matmul(
                                    kv_ps[p][ro:ro + DEP, 1, :],
                                    vg[i][:, p * P + ro:p * P + ro + DEP],
                                    eft[:, 1, par, i, :],
                                    start=False,
                                    stop=(g == NG - 1 and i == 3),
                                    skip_group_check=True)

                # evict kp to bf16 stationaries; transpose vp pairs into vp2
                for p in range(4):
                    nc.vector.tensor_copy(kpP[p][:], kv_ps[p][:, 0, :])
                    vpS = pkv.tile([P, R], BF16, tag="vpS", name="vpS")
                    nc.vector.tensor_copy(vpS[:], kv_ps[p][:, 1, :])
                    if dbg and p == 0:
                        nc.sync.dma_start(dkpP, kpP[0][:])
                        nc.sync.dma_start(dqT, qT[0][:])
                    for rc in range(2):
                        vt = psA.tile([P, P], BF16, tag="vt", name="vt")
                        nc.tensor.transpose(vt[:], vpS[:, rc * P:(rc + 1) * P],
                                            ident[:])
                        for par in range(2):
                            h = 2 * p + par
                            ro = DEP * par
                            nc.vector.tensor_copy(
                                vp2[h][rc][:, ro:ro + DEP], vt[:, ro:ro + DEP])

            if phase == "ab":
                with tc.tile_pool(name="pdump", bufs=1) as pd_, \
                     tc.tile_pool(name="psd", bufs=1, space="PSUM") as _psd:
                    dump = pd_.tile([P, R], F32, tag="dump", name="dump")
                    nc.vector.tensor_copy(dump[:], kpP[0][:])
                    nc.sync.dma_start(y[0:P, 0:R], dump[:])
                continue

            # ================= PHASE C..G =================
            with tc.tile_pool(name="psc", bufs=4, space="PSUM") as psc, \
                 tc.tile_pool(name="psf", bufs=2, space="PSUM") as psf, \
                 tc.tile_pool(name="psb", bufs=1, space="PSUM") as psb, \
                 tc.tile_pool(name="psy", bufs=1, space="PSUM") as psy, \
                 tc.tile_pool(name="pexp", bufs=6) as pexp, \
                 tc.tile_pool(name="pstag", bufs=10) as pstag, \
                 tc.tile_pool(name="pbc", bufs=4) as pbc, \
                 tc.tile_pool(name="pys", bufs=3) as pys, \
                 tc.tile_pool(name="pcs", bufs=2) as pcs:
                for s in range(NS):
                    c0 = s * SN
                    csR = pcs.tile([P, 4, SN], BF16, tag="csR", name="csR")
                    for p in range(4):
                        stags = {}
                        for par in range(2):
                            h = 2 * p + par
                            ro = DEP * par
                            expT = [pexp.tile([P, SN], BF16, tag="expT",
                                              name="expT") for rc in range(2)]
                            for rc in range(2):
                                scp = psc.tile([P, SN], F32, tag="sc", name="sc")
                                nc.tensor.matmul(
                                    scp[:],
                                    kpP[p][ro:ro + DEP, rc * P:(rc + 1) * P],
                                    qT[p][ro:ro + DEP, c0:c0 + SN],
                                    start=True, stop=True)
                                nc.scalar.activation(
                                    expT[rc][:], scp[:], EXPF,
                                    scale=float(1.0 / np.sqrt(np.float32(DEP))))
                            fop = psf.tile([P, SN], F32, tag="fo", name="fo")
                            for rc in range(2):
                                nc.tensor.matmul(
                                    fop[:], vp2[h][rc][:], expT[rc][:],
                                    start=(rc == 0), stop=(rc == 1))
                            stag = pstag.tile([P, SN], BF16, tag="stag",
                                              name="stag")
                            nc.vector.tensor_copy(stag[:], fop[:])
                            stags[par] = stag
                            if dbg and s == 0 and p == 0:
                                nc.sync.dma_start(dstag[par], stag[:])
                        # broadcast the two sum rows across partitions
                        # (accumulation-group one-hot matmuls), reciprocal
                        # fused with the eviction
                        bcp = psb.tile([P, SN], F32, tag="bc", name="bc")
                        nc.tensor.matmul(bcp[:], S0[:], stags[0][:],
                                         start=True, stop=False)
                        nc.tensor.matmul(bcp[:], S1[:], stags[1][:],
                                         start=False, stop=True)
                        bcs = pbc.tile([P, SN], F32, tag="bcs", name="bcs")
                        nc.vector.reciprocal(bcs[:], bcp[:])
                        if dbg and s == 0 and p == 0:
                            nc.sync.dma_start(dbcs, bcs[:])
                        for par in range(2):
                            ro = DEP * par
                            nc.gpsimd.tensor_mul(
                                csR[ro:ro + DEP, p, :],
                                stags[par][ro:ro + DEP, :],
                                bcs[ro:ro + DEP, :])
                    if dbg and s == 0:
                        nc.sync.dma_start(dcsR, csR[:])
                    ys = pys.tile([P, 4, D], F32, tag="ys", name="ys")
                    for j in range(4):
                        yp = psy.tile([P, D], F32, tag="y", name="y")
                        for c in range(4):
                            nc.tensor.matmul(
                                yp[:], csR[:, c, j * P:(j + 1) * P], wd_t[c][:],
                                start=(c == 0), stop=(c == 3))
                        nc.vector.tensor_copy(ys[:, j, :], yp[:])
                    nc.sync.dma_start(
                        y[c0:c0 + SN, :].rearrange("(j p) d -> p j d", p=P), ys[:])

    nc.compile()
    _cache[key] = nc
    return nc


_BF = mybir.dt.np(BF16)


def _pack_ef(E, F):
    # [H, N, R] x2 -> [NG, 4, P, ef(2), par(2), 4, R] with n = g*512 + i*128 + p
    # and h = 2*p + par
    out = np.empty((NG, 4, P, 2, 2, 4, R), _BF)
    for ef, t in ((0, E), (1, F)):
        t = np.asarray(t, np.float32).astype(_BF)
        # [H, N, R] -> [H, NG, 4(i), P, R] -> [H, NG, P, 4, R]
        tp = t.reshape(H, NG, 4, P, R).transpose(0, 1, 3, 2, 4)
        for p in range(4):
            for par in range(2):
                out[:, p, :, ef, par] = tp[2 * p + par]
    return np.ascontiguousarray(out)


def make_in_maps(x, wq, wk, wv, E, F, w_dense, b_dense):
    xT = np.asarray(x, dtype=np.float32).astype(_BF).transpose(0, 2, 1)
    xT = np.ascontiguousarray(xT)  # [B, D, N]
    consts = {
        "wq": np.ascontiguousarray(np.asarray(wq, np.float32)).astype(_BF),
        "wk": np.ascontiguousarray(np.asarray(wk, np.float32)).astype(_BF),
        "wv": np.ascontiguousarray(np.asarray(wv, np.float32)).astype(_BF),
        "wd": np.ascontiguousarray(np.asarray(w_dense, np.float32)).astype(_BF),
        "EF": _pack_ef(E, F),
        "ident": np.eye(P, dtype=np.float32).astype(_BF),
    }
    return [{"x": xT[b], **consts} for b in range(B)]


def kernel(x, wq, wk, wv, E, F, w_dense, b_dense):
    nc = build_program()
    in_maps = make_in_maps(x, wq, wk, wv, E, F, w_dense, b_dense)
    res = run_bass_kernel_spmd(nc, in_maps, list(range(B)))
    out = np.stack([res.results[b]["y"] for b in range(B)], axis=0)
    out = out.astype(np.float32) + np.asarray(b_dense, np.float32)[None, None, :]
    return out
